# revision 12
# baseline (speedup 1.0000x reference)
"""DiT block kernel for 8 trn2 NeuronCores.

Sharding: core c -> (batch b=c//2, query-token half h=c%2). Each core
computes the full block for its 512 query tokens (K/V for all 1024
tokens of its batch replicated within the pair) -> zero collectives.

v2: fp8e4 + DoubleRow matmuls for qkv/proj/fc1/fc2/softmax-sums/attnV
(weights host-scaled by WS=32, descaled at PSUM evacuation); scores stay
fp16; LN stats in fp16; modulation vectors folded on the host into the
input prep (tiny matvecs, 0.02% of model FLOPs). Activations are
feature-major ([D on partitions, tokens on free]).
"""
import numpy as np

import concourse.bass as bass
import concourse.tile as tile
import concourse.mybir as mybir
from concourse.bass_utils import run_bass_kernel_spmd
from concourse.vector_clock import ScopedClock
from concourse.alu_op_type import AluOpType

dt = mybir.dt
AF = mybir.ActivationFunctionType
DR = mybir.MatmulPerfMode.DoubleRow

P = 128
B, NT, D, H = 4, 1024, 1024, 16
DH = D // H            # 64
DFF = 4 * D            # 4096
KC = D // P            # 8
LT = NT // 2           # 512 local query tokens
GATE = 0.1
EPS = 1e-5
EXP_SCALE = DH ** -0.5
EXP_BIAS = -1.5        # keeps exp outputs inside fp8e4m3 range; cancels
WS = 32.0              # fp8 weight pre-scale
IWS = 1.0 / WS


class SplitDrainTileContext(tile.TileContext):
    """Tail drain in this walrus build holds few sync waits; spill the
    rest onto chained SP nops (runs before the sem-clear barrier, so
    semantics are preserved)."""

    MAX_TAIL_WAITS = 1

    def _drain_and_barrier(self, tick_clock, wait_clock):
        drain_inst = self.nc.sync.drain()
        wait_clock.add_sem_waits(
            drain_inst.ins, ScopedClock({None: tick_clock.global_clock})
        )
        si = drain_inst.ins.sync_info
        waits = list(si.on_wait) if si else []
        if len(waits) > self.MAX_TAIL_WAITS:
            drain_inst.ins.sync_info = mybir.SyncInfo(
                on_wait=waits[: self.MAX_TAIL_WAITS],
                on_update=list(si.on_update) if si else [],
            )
            rest = waits[self.MAX_TAIL_WAITS:]
            for i in range(0, len(rest), self.MAX_TAIL_WAITS):
                nop = self.nc.sync.nop()
                nop.ins.sync_info = mybir.SyncInfo(
                    on_wait=rest[i : i + self.MAX_TAIL_WAITS], on_update=[]
                )
        self.nc.all_engine_barrier()
        assert self.sems is not None
        popped = self.nc._tile_sem_poison_stack.pop()
        assert popped is self._sem_poison
        self.nc.clear_and_free_semaphores(list(self.sems.allocated().values()))
        self.nc.all_engine_barrier()


def _legalize_waits(nc, max_waits=1):
    """This walrus build accepts at most one sync wait per instruction.
    Move surplus waits onto same-engine NoOps inserted just before the
    offending instruction (engine FIFO order preserves semantics)."""
    fix = 0
    for bb in nc.main_func.blocks:
        insts = list(bb.instructions)
        out = []
        for inst in insts:
            si = inst.sync_info
            waits = list(si.on_wait) if si else []
            if len(waits) > max_waits:
                keep = waits[-max_waits:]
                for w in waits[:-max_waits]:
                    nop = mybir.InstNoOp(name=f"I-wfix{fix}")
                    fix += 1
                    nop.engine = inst.engine
                    nop.sync_info = mybir.SyncInfo(on_wait=[w], on_update=[])
                    out.append(nop)
                inst.sync_info = mybir.SyncInfo(
                    on_wait=keep, on_update=list(si.on_update) if si else [])
            out.append(inst)
        if len(out) != len(insts):
            bb.instructions = out
    return fix


def _build():
    nc = bass.Bass(target_bir_lowering=False, debug=False,
                   dynamic_dma_scratch_size=2048)
    f32, f16, f8 = dt.float32, dt.float16, dt.float8e4

    xt16 = nc.dram_tensor("xt16", [D, NT], f16, kind="ExternalInput")
    vecsf = nc.dram_tensor("vecsf", [P, 8 * KC], f32, kind="ExternalInput")
    qkvw8 = nc.dram_tensor("qkvw8", [D, 3 * D], f8, kind="ExternalInput")
    qkvbf = nc.dram_tensor("qkvbf", [P, 16], f32, kind="ExternalInput")
    bvrow = nc.dram_tensor("bvrow", [1, D], f16, kind="ExternalInput")
    projw8 = nc.dram_tensor("projw8", [D, D], f8, kind="ExternalInput")
    projbrow = nc.dram_tensor("projbrow", [1, D], f16, kind="ExternalInput")
    fc1w8 = nc.dram_tensor("fc1w8", [D, DFF], f8, kind="ExternalInput")
    fc1bf = nc.dram_tensor("fc1bf", [P, 32], f32, kind="ExternalInput")
    fc2w8 = nc.dram_tensor("fc2w8", [D, DFF], f8, kind="ExternalInput")  # host-rearranged
    fc2brow = nc.dram_tensor("fc2brow", [1, D], f16, kind="ExternalInput")
    outt = nc.dram_tensor("outt", [D, LT], f32, kind="ExternalOutput")

    with SplitDrainTileContext(nc) as tc:
        with tc.tile_pool(name="cp", bufs=1) as cp, \
             tc.tile_pool(name="ar", bufs=1) as ar, \
             tc.tile_pool(name="rot", bufs=4) as rot, \
             tc.tile_pool(name="psA", bufs=2, space="PSUM") as psA, \
             tc.tile_pool(name="psB", bufs=3, space="PSUM") as psB:

            def pp():    # [P, 512] f32 psum, 2 rotating banks
                return psA.tile([P, 512], f32, tag="pp", name="pp")

            def pbig():  # [P, 1024] f32 psum, 3 rotating 2-bank tiles
                return psB.tile([P, 1024], f32, tag="big", name="big")

            ones16 = cp.tile([P, P], f16, tag="ones16")
            nc.vector.memset(ones16[:], 1.0)
            ones8 = cp.tile([P, 2, P], f8, tag="ones8")
            nc.vector.memset(ones8[:], 1.0)
            onesrow = cp.tile([1, LT], f16, tag="onesrow")
            nc.vector.memset(onesrow[:], 1.0)
            expb = cp.tile([P, 1], f32, tag="expb")
            nc.vector.memset(expb[:], EXP_BIAS)

            # ---- resident small inputs ----
            vecs = cp.tile([P, 8, KC], f32, tag="vecs")
            nc.sync.dma_start(vecs[:], vecsf.rearrange("p (w c) -> p w c", c=KC))
            qkvbt = cp.tile([P, 16], f32, tag="qkvbt")
            nc.sync.dma_start(qkvbt[:], qkvbf[:])
            fc1bt = cp.tile([P, 32], f32, tag="fc1bt")
            nc.sync.dma_start(fc1bt[:], fc1bf[:])
            bvt = cp.tile([1, D], f16, tag="bvt")
            nc.sync.dma_start(bvt[:], bvrow[:])
            pbrow = cp.tile([1, D], f16, tag="pbrow")
            nc.sync.dma_start(pbrow[:], projbrow[:])
            f2brow = cp.tile([1, D], f16, tag="f2brow")
            nc.sync.dma_start(f2brow[:], fc2brow[:])

            xf = ar.tile([P, KC, NT], f16, tag="X16")
            nc.sync.dma_start(xf[:], xt16.rearrange("(c p) t -> p c t", p=P))
            qkA = ar.tile([P, KC, 2 * D], f8, tag="W2")
            nc.sync.dma_start(qkA[:],
                              qkvw8[:, 0:2 * D].rearrange("(c p) m -> p c m", p=P))
            vW = ar.tile([P, KC, D], f8, tag="VW")
            nc.sync.dma_start(vW[:],
                              qkvw8[:, 2 * D:3 * D].rearrange("(c p) m -> p c m", p=P))

            def layernorm(src16, ntok, scol, bcol, ncol, out8):
                """out8[:, j] = (LN(src)*scale+shift) in fp8e4; f16 stats."""
                halves = ntok // 512
                pss = pbig()
                psq = pbig()
                for j in range(KC):
                    s16 = rot.tile([P, NT], f16, tag="S16", bufs=3, name="s16")
                    nc.vector.tensor_tensor(s16[:, 0:ntok],
                                            src16[:, j, 0:ntok],
                                            src16[:, j, 0:ntok],
                                            AluOpType.mult)
                    for nh in range(halves):
                        sl = slice(nh * 512, (nh + 1) * 512)
                        nc.tensor.matmul(pss[:, sl], ones16[:],
                                         src16[:, j, sl],
                                         start=(j == 0), stop=(j == KC - 1),
                                         skip_group_check=True)
                        nc.tensor.matmul(psq[:, sl], ones16[:],
                                         s16[:, sl],
                                         start=(j == 0), stop=(j == KC - 1),
                                         skip_group_check=True)

                def st(name):
                    return rot.tile([P, ntok], f16, tag=f"ST{ntok}", bufs=7,
                                    name=name)

                mu = st("mu")
                nc.vector.tensor_scalar_mul(mu[:], pss[:, 0:ntok], 1.0 / D)
                msq = st("msq")
                nc.vector.tensor_scalar(msq[:], psq[:, 0:ntok], 1.0 / D, EPS,
                                        AluOpType.mult, AluOpType.add)
                mu2 = st("mu2")
                nc.vector.tensor_tensor(mu2[:], mu[:], mu[:], AluOpType.mult)
                var = st("var")
                nc.vector.tensor_tensor(var[:], msq[:], mu2[:],
                                        AluOpType.subtract)
                rvar = st("rvar")
                with nc.allow_low_precision(reason="LN rstd fine in f16"):
                    nc.vector.reciprocal(rvar[:], var[:])
                A = st("A")
                nc.scalar.activation(A[:], rvar[:], AF.Sqrt)
                M = st("M")
                nc.vector.tensor_tensor(M[:], mu[:], A[:], AluOpType.mult)
                for j in range(KC):
                    u = rot.tile([P, NT], f16, tag="U16", bufs=3, name="u")
                    nc.vector.tensor_tensor(u[:, 0:ntok], src16[:, j, 0:ntok],
                                            A[:], AluOpType.mult)
                    w = rot.tile([P, NT], f16, tag="U16", bufs=3, name="w")
                    nc.vector.tensor_scalar(w[:, 0:ntok], u[:, 0:ntok],
                                            vecs[:, scol, j:j + 1],
                                            vecs[:, bcol, j:j + 1],
                                            AluOpType.mult, AluOpType.add)
                    nc.vector.scalar_tensor_tensor(out8[:, j], M[:],
                                                   vecs[:, ncol, j:j + 1],
                                                   w[:, 0:ntok],
                                                   AluOpType.mult,
                                                   AluOpType.add)

            # ---- LN1 + modulate (all 1024 tokens) -> y8 ----
            y8 = ar.tile([P, KC, NT], f8, tag="Y8")
            layernorm(xf, NT, 0, 1, 6, y8)

            # ---- qkv (DoubleRow fp8) ----
            # q8/k8 layout (weight columns host-permuted): tile (gg, j) has
            # partition 32*s+p <-> head 4*gg+s, dh j*32+p. Scores for head s
            # run as K=32 DoubleRow matmuls on partition quarter s (row
            # placement composes with DR; col placement does not).
            # v8b holds V in block-diagonal head-pair blocks:
            # [:, tok-chunk, pair g, 0, 0:64]=v_{2g}, [:, .., 1, 64:128]=
            # v_{2g+1}, zeros elsewhere -> one M=128 DoubleRow matmul per
            # chunk computes both heads of attn@V at PE column group 0
            # (DR outputs must start at column group 0).
            q8 = ar.tile([P, 4, 2, LT], f8, tag="Q8")
            k8 = ar.tile([P, 4, 2, NT], f8, tag="K16")
            v8b = ar.tile([P, KC, KC, 2, P], f8, tag="V8B")
            nc.gpsimd.memset(v8b[:], 0.0)
            for mt in range(KC):  # q, local tokens
                pq = pp()
                for kp in range(0, KC, 2):
                    nc.tensor.matmul(pq[:], qkA[:, kp:kp + 2, mt * P:(mt + 1) * P],
                                     y8[:, kp:kp + 2, 0:LT],
                                     start=(kp == 0), stop=(kp == KC - 2),
                                     perf_mode=DR)
                nc.vector.tensor_scalar(q8[:, mt // 2, mt % 2, :], pq[:],
                                        IWS, qkvbt[:, mt:mt + 1],
                                        AluOpType.mult, AluOpType.add)
            for mt in range(KC):  # k, all tokens
                for nh in range(2):
                    pk = pp()
                    for kp in range(0, KC, 2):
                        nc.tensor.matmul(
                            pk[:], qkA[:, kp:kp + 2, D + mt * P:D + (mt + 1) * P],
                            y8[:, kp:kp + 2, nh * 512:(nh + 1) * 512],
                            start=(kp == 0), stop=(kp == KC - 2),
                            perf_mode=DR)
                    nc.vector.tensor_scalar(
                        k8[:, mt // 2, mt % 2, nh * 512:(nh + 1) * 512],
                        pk[:], IWS, qkvbt[:, 8 + mt:9 + mt],
                        AluOpType.mult, AluOpType.add)
            pb = pbig()  # v bias replicated across partitions
            for nh in range(2):
                nc.tensor.matmul(pb[:, nh * 512:(nh + 1) * 512], ones16[0:1, :],
                                 bvt[:, nh * 512:(nh + 1) * 512],
                                 start=True, stop=True, skip_group_check=True)
            bvrep = rot.tile([P, KC, P], f16, tag="BV", bufs=1, name="bvrep")
            nc.vector.tensor_copy(bvrep[:], pb[:])
            for tt in range(KC):  # v rows = tokens (all)
                pv = psB.tile([P, KC, P], f32, tag="big", name="pv")
                for kp in range(0, KC, 2):
                    for nh in range(2):
                        nc.tensor.matmul(
                            pv[:, nh * 4:(nh + 1) * 4, :],
                            y8[:, kp:kp + 2, tt * P:(tt + 1) * P],
                            vW[:, kp:kp + 2, nh * 512:(nh + 1) * 512],
                            start=(kp == 0), stop=(kp == KC - 2),
                            perf_mode=DR, skip_group_check=True)
                # scatter even/odd head features into the diagonal blocks
                nc.vector.scalar_tensor_tensor(v8b[:, tt, :, 0, 0:DH],
                                               pv[:, :, 0:DH], IWS,
                                               bvrep[:, :, 0:DH],
                                               AluOpType.mult, AluOpType.add)
                nc.vector.scalar_tensor_tensor(v8b[:, tt, :, 1, DH:P],
                                               pv[:, :, DH:P], IWS,
                                               bvrep[:, :, DH:P],
                                               AluOpType.mult, AluOpType.add)

            # ---- attention, head pair (2g, 2g+1) per feature tile g ----
            attn8 = ar.tile([P, KC, LT], f8, tag="AT8")
            pw = ar.tile([P, KC, D], f8, tag="PW")
            nc.sync.dma_start(pw[:], projw8.rearrange("(c p) m -> p c m", p=P))
            for g in range(KC):
                eg = ar.tile([P, KC, 2, 512], f8, tag="EG", bufs=2)
                for c in range(KC):
                    psc = pbig()
                    nc.tensor.matmul(psc[:, 0:512],
                                     k16[0:DH, g, c * P:(c + 1) * P],
                                     q16[0:DH, g, :], start=True, stop=True,
                                     skip_group_check=True)
                    nc.tensor.matmul(psc[:, 512:1024],
                                     k16[DH:P, g, c * P:(c + 1) * P],
                                     q16[DH:P, g, :], start=True, stop=True,
                                     skip_group_check=True)
                    nc.scalar.activation(eg[:, c], psc[:], AF.Exp,
                                         scale=EXP_SCALE, bias=expb[:])
                pse = pbig()
                for cp2 in range(0, KC, 2):
                    for nh in range(2):
                        sl = slice(nh * 512, (nh + 1) * 512)
                        nc.tensor.matmul(pse[:, sl], ones8[:],
                                         eg[:, cp2:cp2 + 2, nh, :],
                                         start=(cp2 == 0), stop=(cp2 == KC - 2),
                                         perf_mode=DR, skip_group_check=True)
                recip = rot.tile([P, NT], f16, tag="RC", bufs=2, name="recip")
                with nc.allow_low_precision(reason="softmax recip fine in f16"):
                    nc.vector.reciprocal(recip[:], pse[:])
                pav = pp()
                for c in range(KC):
                    nc.tensor.matmul(pav[:], v8b[:, c, g], eg[:, c],
                                     start=(c == 0), stop=(c == KC - 1),
                                     perf_mode=DR)
                nc.vector.tensor_tensor(attn8[0:DH, g], pav[0:DH, :],
                                        recip[0:DH, 0:512], AluOpType.mult)
                nc.vector.tensor_tensor(attn8[DH:P, g], pav[DH:P, :],
                                        recip[DH:P, 512:1024], AluOpType.mult)

            # ---- proj + gated residual ----
            f1a = ar.tile([P, KC, 2 * D], f8, tag="W2")  # reuses qkA slot
            nc.sync.dma_start(f1a[:],
                              fc1w8[:, 0:2 * D].rearrange("(c p) m -> p c m", p=P))
            x2 = ar.tile([P, KC, LT], f16, tag="Q16")    # reuses q16 slot
            for mt in range(KC):
                pj = pp()
                for kp in range(0, KC, 2):
                    nc.tensor.matmul(pj[:], pw[:, kp:kp + 2, mt * P:(mt + 1) * P],
                                     attn8[:, kp:kp + 2, :],
                                     start=(kp == 0), stop=False,
                                     perf_mode=DR)
                nc.tensor.matmul(pj[:], pbrow[:, mt * P:(mt + 1) * P],
                                 onesrow[:], start=False, stop=True)
                nc.vector.scalar_tensor_tensor(x2[:, mt], pj[:],
                                               vecs[:, 2, mt:mt + 1],
                                               xf[:, mt, 0:LT],
                                               AluOpType.mult, AluOpType.add)

            # ---- LN2 + modulate (local tokens) -> z8 ----
            z8 = ar.tile([P, KC, LT], f8, tag="AT8")     # reuses attn8 slot
            layernorm(x2, LT, 3, 4, 7, z8)

            # ---- fc1 + gelu (DoubleRow fp8) ----
            h8 = ar.tile([P, 32, LT], f8, tag="X16")     # reuses xf slot
            f1b = ar.tile([P, KC, 2 * D], f8, tag="K16")  # reuses k16 slot
            nc.sync.dma_start(f1b[:],
                              fc1w8[:, 2 * D:4 * D].rearrange("(c p) m -> p c m", p=P))

            def fc1_block(wt, mg0, nmt):
                for mt in range(nmt):
                    mg = mg0 + mt
                    ph = pp()
                    for kp in range(0, KC, 2):
                        nc.tensor.matmul(ph[:], wt[:, kp:kp + 2, mt * P:(mt + 1) * P],
                                         z8[:, kp:kp + 2, :],
                                         start=(kp == 0), stop=(kp == KC - 2),
                                         perf_mode=DR)
                    nc.scalar.activation(h8[:, mg], ph[:], AF.Gelu,
                                         bias=fc1bt[:, mg:mg + 1], scale=IWS)

            fc1_block(f1a, 0, 16)
            fc1_block(f1b, 16, 16)

            # ---- fc2 + gated residual + store (DoubleRow fp8) ----
            for mt in range(KC):
                f2col = ar.tile([P, 32, P], f8, tag="F2C", bufs=3)
                nc.sync.dma_start(
                    f2col[:],
                    fc2w8[mt * P:(mt + 1) * P, :]
                    .rearrange("p (c m) -> p c m", m=P))
                pz = pp()
                for kp in range(0, 32, 2):
                    nc.tensor.matmul(pz[:], f2col[:, kp:kp + 2, :],
                                     h8[:, kp:kp + 2, :],
                                     start=(kp == 0), stop=False,
                                     perf_mode=DR)
                nc.tensor.matmul(pz[:], f2brow[:, mt * P:(mt + 1) * P],
                                 onesrow[:], start=False, stop=True)
                ot = rot.tile([P, LT], f32, tag="OT", bufs=2)
                nc.vector.scalar_tensor_tensor(ot[:], pz[:],
                                               vecs[:, 5, mt:mt + 1],
                                               x2[:, mt, :],
                                               AluOpType.mult, AluOpType.add)
                nc.sync.dma_start(outt[mt * P:(mt + 1) * P, :], ot[:])

    _legalize_waits(nc)
    return nc


_NC_CACHE = {}


def _get_nc():
    if "nc" not in _NC_CACHE:
        _NC_CACHE["nc"] = _build()
    return _NC_CACHE["nc"]


def _feat(v, cols):
    """[D*]-vector -> feature-major [128, cols] (col j = chunk j)."""
    return np.ascontiguousarray(np.asarray(v, np.float32).reshape(cols, P).T)


def make_in_maps(x, cond, g1_w, g1_b, b1_w, b1_b, a1_w, a1_b,
                 g2_w, g2_b, b2_w, b2_b, a2_w, a2_b,
                 ln1_g, ln1_b, ln2_g, ln2_b,
                 qkv_w, qkv_b, proj_w, proj_b,
                 fc1_w, fc1_b, fc2_w, fc2_b):
    f32 = np.float32
    f16 = np.float16
    f8 = dt.np(dt.float8e4)
    x = np.asarray(x, f32)
    cond = np.asarray(cond, f32)

    def w8(w):
        return (np.asarray(w, f32) * WS).astype(f8)

    shared = {
        "qkvw8": w8(qkv_w),
        "qkvbf": np.hstack([_feat(np.asarray(qkv_b, f32)[0:D], KC),
                            _feat(np.asarray(qkv_b, f32)[D:2 * D], KC)]),
        "bvrow": np.asarray(qkv_b, f16)[None, 2 * D:3 * D],
        "projw8": w8(proj_w),
        "projbrow": (np.asarray(proj_b, f32) * WS).astype(f16)[None, :],
        "fc1w8": w8(fc1_w),
        "fc1bf": _feat(np.asarray(fc1_b, f32), 32),
        # [mt*128+p, kc*128+m] = fc2_w[kc*128+p, mt*128+m]: contiguous
        # per-mt loads of the feature-major lhsT tiles
        "fc2w8": np.ascontiguousarray(
            w8(fc2_w).reshape(32, P, KC, P)
            .transpose(2, 1, 0, 3).reshape(D, DFF)),
        "fc2brow": (np.asarray(fc2_b, f32) * WS).astype(f16)[None, :],
    }

    # host-folded modulation vectors (25M MACs, 0.02% of model FLOPs)
    g1 = cond @ np.asarray(g1_w, f32) + np.asarray(g1_b, f32)
    b1 = cond @ np.asarray(b1_w, f32) + np.asarray(b1_b, f32)
    a1 = np.tanh(cond @ np.asarray(a1_w, f32) + np.asarray(a1_b, f32)) * GATE
    g2 = cond @ np.asarray(g2_w, f32) + np.asarray(g2_b, f32)
    b2 = cond @ np.asarray(b2_w, f32) + np.asarray(b2_b, f32)
    a2 = np.tanh(cond @ np.asarray(a2_w, f32) + np.asarray(a2_b, f32)) * GATE
    s1 = (1.0 + g1) * np.asarray(ln1_g, f32)[None, :]
    t1 = (1.0 + g1) * np.asarray(ln1_b, f32)[None, :] + b1
    s2 = (1.0 + g2) * np.asarray(ln2_g, f32)[None, :]
    t2 = (1.0 + g2) * np.asarray(ln2_b, f32)[None, :] + b2

    in_maps = []
    for c in range(8):
        b, h = c // 2, c % 2
        xb = x[b].T  # [D, NT]
        perm = np.concatenate([np.arange(h * LT, (h + 1) * LT),
                               np.arange((1 - h) * LT, (2 - h) * LT)])
        m = dict(shared)
        m["xt16"] = np.ascontiguousarray(xb[:, perm]).astype(f16)
        m["vecsf"] = np.hstack([
            _feat(s1[b], KC), _feat(t1[b], KC), _feat(a1[b] * IWS, KC),
            _feat(s2[b], KC), _feat(t2[b], KC), _feat(a2[b] * IWS, KC),
            _feat(-s1[b], KC), _feat(-s2[b], KC)])
        in_maps.append(m)
    return in_maps


def kernel(**inputs):
    nc = _get_nc()
    in_maps = make_in_maps(**inputs)
    res = run_bass_kernel_spmd(nc, in_maps, list(range(8)))
    out = np.empty((B, NT, D), np.float32)
    for c in range(8):
        b, h = c // 2, c % 2
        out[b, h * LT:(h + 1) * LT, :] = res.results[c]["outt"].T
    return out


# revision 24
# speedup vs baseline: 1.3682x; 1.3682x over previous
"""DiT block kernel for 8 trn2 NeuronCores.

Sharding: core c -> (batch b=c//2, query-token half h=c%2). Each core
computes the full block for its 512 query tokens (K/V for all 1024
tokens of its batch replicated within the pair) -> zero collectives.

v2: fp8e4 + DoubleRow matmuls for qkv/proj/fc1/fc2/softmax-sums/attnV
(weights host-scaled by WS=32, descaled at PSUM evacuation); scores stay
fp16; LN stats in fp16; modulation vectors folded on the host into the
input prep (tiny matvecs, 0.02% of model FLOPs). Activations are
feature-major ([D on partitions, tokens on free]).
"""
import numpy as np

import concourse.bass as bass
import concourse.tile as tile
import concourse.mybir as mybir
from concourse.bass_utils import run_bass_kernel_spmd
from concourse.vector_clock import ScopedClock
from concourse.alu_op_type import AluOpType

dt = mybir.dt
AF = mybir.ActivationFunctionType
DR = mybir.MatmulPerfMode.DoubleRow

P = 128
B, NT, D, H = 4, 1024, 1024, 16
DH = D // H            # 64
DFF = 4 * D            # 4096
KC = D // P            # 8
LT = NT // 2           # 512 local query tokens
GATE = 0.1
EPS = 1e-5
EXP_SCALE = DH ** -0.5
EXP_BIAS = -1.5        # keeps exp outputs inside fp8e4m3 range; cancels
WS = 32.0              # fp8 weight pre-scale
IWS = 1.0 / WS
# Schraudolph fast-exp constants (DVE bit-trick): i32 = A*scores + B, then
# bitcast to f32 approximates exp(EXP_SCALE*scores + EXP_BIAS) to ~2-4%.
SCH_A = 8388608.0 * 1.4426950408889634 * EXP_SCALE
SCH_B = 8388608.0 * (127.0 + 1.4426950408889634 * EXP_BIAS) - 366393.0


class SplitDrainTileContext(tile.TileContext):
    """Tail drain in this walrus build holds few sync waits; spill the
    rest onto chained SP nops (runs before the sem-clear barrier, so
    semantics are preserved)."""

    MAX_TAIL_WAITS = 1

    def _drain_and_barrier(self, tick_clock, wait_clock):
        drain_inst = self.nc.sync.drain()
        wait_clock.add_sem_waits(
            drain_inst.ins, ScopedClock({None: tick_clock.global_clock})
        )
        si = drain_inst.ins.sync_info
        waits = list(si.on_wait) if si else []
        if len(waits) > self.MAX_TAIL_WAITS:
            drain_inst.ins.sync_info = mybir.SyncInfo(
                on_wait=waits[: self.MAX_TAIL_WAITS],
                on_update=list(si.on_update) if si else [],
            )
            rest = waits[self.MAX_TAIL_WAITS:]
            for i in range(0, len(rest), self.MAX_TAIL_WAITS):
                nop = self.nc.sync.nop()
                nop.ins.sync_info = mybir.SyncInfo(
                    on_wait=rest[i : i + self.MAX_TAIL_WAITS], on_update=[]
                )
        self.nc.all_engine_barrier()
        assert self.sems is not None
        popped = self.nc._tile_sem_poison_stack.pop()
        assert popped is self._sem_poison
        self.nc.clear_and_free_semaphores(list(self.sems.allocated().values()))
        self.nc.all_engine_barrier()


def _legalize_waits(nc, max_waits=1):
    """This walrus build accepts at most one sync wait per instruction.
    Move surplus waits onto same-engine NoOps inserted just before the
    offending instruction (engine FIFO order preserves semantics)."""
    fix = 0
    for bb in nc.main_func.blocks:
        insts = list(bb.instructions)
        out = []
        for inst in insts:
            si = inst.sync_info
            waits = list(si.on_wait) if si else []
            if len(waits) > max_waits:
                keep = waits[-max_waits:]
                for w in waits[:-max_waits]:
                    nop = mybir.InstNoOp(name=f"I-wfix{fix}")
                    fix += 1
                    nop.engine = inst.engine
                    nop.sync_info = mybir.SyncInfo(on_wait=[w], on_update=[])
                    out.append(nop)
                inst.sync_info = mybir.SyncInfo(
                    on_wait=keep, on_update=list(si.on_update) if si else [])
            out.append(inst)
        if len(out) != len(insts):
            bb.instructions = out
    return fix


def _build():
    nc = bass.Bass(target_bir_lowering=False, debug=False,
                   dynamic_dma_scratch_size=2048)
    f32, f16, f8 = dt.float32, dt.float16, dt.float8e4

    xt16 = nc.dram_tensor("xt16", [D, NT], f16, kind="ExternalInput")
    vecsf = nc.dram_tensor("vecsf", [P, 8 * KC], f32, kind="ExternalInput")
    qkvw8 = nc.dram_tensor("qkvw8", [D, 3 * D], f8, kind="ExternalInput")
    qkvbf = nc.dram_tensor("qkvbf", [P, 16], f32, kind="ExternalInput")
    bvrow = nc.dram_tensor("bvrow", [1, D], f16, kind="ExternalInput")
    projw8 = nc.dram_tensor("projw8", [D, D], f8, kind="ExternalInput")
    projbrow = nc.dram_tensor("projbrow", [1, D], f16, kind="ExternalInput")
    fc1w8 = nc.dram_tensor("fc1w8", [D, DFF], f8, kind="ExternalInput")
    fc1bf = nc.dram_tensor("fc1bf", [P, 32], f32, kind="ExternalInput")
    fc2w8 = nc.dram_tensor("fc2w8", [D, DFF], f8, kind="ExternalInput")  # host-rearranged
    fc2brow = nc.dram_tensor("fc2brow", [1, D], f16, kind="ExternalInput")
    outt = nc.dram_tensor("outt", [D, LT], f32, kind="ExternalOutput")

    with SplitDrainTileContext(nc) as tc:
        with tc.tile_pool(name="cp", bufs=1) as cp, \
             tc.tile_pool(name="ar", bufs=1) as ar, \
             tc.tile_pool(name="rot", bufs=4) as rot, \
             tc.tile_pool(name="psA", bufs=2, space="PSUM") as psA, \
             tc.tile_pool(name="psB", bufs=3, space="PSUM") as psB:

            def pp():    # [P, 512] f32 psum, 2 rotating banks
                return psA.tile([P, 512], f32, tag="pp", name="pp")

            def pbig():  # [P, 1024] f32 psum, 3 rotating 2-bank tiles
                return psB.tile([P, 1024], f32, tag="big", name="big")

            ones16 = cp.tile([P, P], f16, tag="ones16")
            nc.vector.memset(ones16[:], 1.0)
            ones8 = cp.tile([P, 2, P], f8, tag="ones8")
            nc.vector.memset(ones8[:], 1.0)
            # masked ones: slot0 hits out cols 0:64, slot1 cols 64:128 ->
            # softmax DR row-sums land per-head in one [128, 512] tile
            # (rows 0:64 = even head, 64:128 = odd head)
            ones8m = cp.tile([P, 2, P], f8, tag="ones8m")
            nc.vector.memset(ones8m[:], 1.0)
            nc.vector.memset(ones8m[:, 0, DH:P], 0.0)
            nc.vector.memset(ones8m[:, 1, 0:DH], 0.0)
            onesrow = cp.tile([1, LT], f16, tag="onesrow")
            nc.vector.memset(onesrow[:], 1.0)
            expb = cp.tile([P, 1], f32, tag="expb")
            nc.vector.memset(expb[:], EXP_BIAS)

            # ---- resident small inputs ----
            vecs = cp.tile([P, 8, KC], f32, tag="vecs")
            nc.sync.dma_start(vecs[:], vecsf.rearrange("p (w c) -> p w c", c=KC))
            qkvbt = cp.tile([P, 16], f32, tag="qkvbt")
            nc.sync.dma_start(qkvbt[:], qkvbf[:])
            fc1bt = cp.tile([P, 32], f32, tag="fc1bt")
            nc.sync.dma_start(fc1bt[:], fc1bf[:])
            bvt = cp.tile([1, D], f16, tag="bvt")
            nc.sync.dma_start(bvt[:], bvrow[:])
            pbrow = cp.tile([1, D], f16, tag="pbrow")
            nc.sync.dma_start(pbrow[:], projbrow[:])
            f2brow = cp.tile([1, D], f16, tag="f2brow")
            nc.sync.dma_start(f2brow[:], fc2brow[:])

            xf = ar.tile([P, KC, NT], f16, tag="X16")
            xsrc = xt16.rearrange("(c p) t -> p c t", p=P)
            nc.sync.dma_start(xf[:, :, 0:512], xsrc[:, :, 0:512])
            nc.sync.dma_start(xf[:, :, 512:1024], xsrc[:, :, 512:1024])
            qkA = ar.tile([P, KC, 2 * D], f8, tag="W2")
            nc.sync.dma_start(qkA[:],
                              qkvw8[:, 0:2 * D].rearrange("(c p) m -> p c m", p=P))
            vW = ar.tile([P, KC, D], f8, tag="VW")
            nc.sync.dma_start(vW[:],
                              qkvw8[:, 2 * D:3 * D].rearrange("(c p) m -> p c m", p=P))

            def layernorm(src16, t0, t1, scol, bcol, ncol, out8, stag,
                          pss, psq):
                """out8[:, j, t0:t1] = (LN(src)*scale+shift) in fp8e4 for
                token range [t0, t1); f16 stats accumulated in the column
                range [t0, t1) of the shared pss/psq psum tiles."""
                ntok = t1 - t0
                sl = slice(t0, t1)
                for j in range(KC):
                    s16 = rot.tile([P, NT], f16, tag="S16", bufs=3, name="s16")
                    nc.vector.tensor_tensor(s16[:, 0:ntok],
                                            src16[:, j, sl],
                                            src16[:, j, sl],
                                            AluOpType.mult)
                    nc.tensor.matmul(pss[:, sl], ones16[:],
                                     src16[:, j, sl],
                                     start=(j == 0), stop=(j == KC - 1),
                                     skip_group_check=True)
                    nc.tensor.matmul(psq[:, sl], ones16[:],
                                     s16[:, 0:ntok],
                                     start=(j == 0), stop=(j == KC - 1),
                                     skip_group_check=True)

                def st(name):
                    return rot.tile([P, ntok], f16, tag=stag, bufs=7,
                                    name=name)

                mu = st("mu")
                nc.vector.tensor_scalar_mul(mu[:], pss[:, sl], 1.0 / D)
                msq = st("msq")
                nc.vector.tensor_scalar(msq[:], psq[:, sl], 1.0 / D, EPS,
                                        AluOpType.mult, AluOpType.add)
                mu2 = st("mu2")
                nc.vector.tensor_tensor(mu2[:], mu[:], mu[:], AluOpType.mult)
                var = st("var")
                nc.vector.tensor_tensor(var[:], msq[:], mu2[:],
                                        AluOpType.subtract)
                rvar = st("rvar")
                with nc.allow_low_precision(reason="LN rstd fine in f16"):
                    nc.vector.reciprocal(rvar[:], var[:])
                A = st("A")
                nc.scalar.activation(A[:], rvar[:], AF.Sqrt)
                M = st("M")
                nc.vector.tensor_tensor(M[:], mu[:], A[:], AluOpType.mult)
                for j in range(KC):
                    u = rot.tile([P, NT], f16, tag="U16", bufs=3, name="u")
                    nc.vector.tensor_tensor(u[:, 0:ntok], src16[:, j, sl],
                                            A[:], AluOpType.mult)
                    w = rot.tile([P, NT], f16, tag="U16", bufs=3, name="w")
                    nc.scalar.activation(w[:, 0:ntok], u[:, 0:ntok],
                                         AF.Identity,
                                         bias=vecs[:, bcol, j:j + 1],
                                         scale=vecs[:, scol, j:j + 1])
                    nc.vector.scalar_tensor_tensor(out8[:, j, sl], M[:],
                                                   vecs[:, ncol, j:j + 1],
                                                   w[:, 0:ntok],
                                                   AluOpType.mult,
                                                   AluOpType.add)

            # ---- LN1 + modulate (all 1024 tokens) -> y8, half-pipelined ----
            y8 = ar.tile([P, KC, NT], f8, tag="Y8")
            pss1 = pbig()
            psq1 = pbig()
            layernorm(xf, 0, 512, 0, 1, 6, y8, "STa", pss1, psq1)
            layernorm(xf, 512, 1024, 0, 1, 6, y8, "STb", pss1, psq1)

            # ---- qkv (DoubleRow fp8) ----
            # q8/k8 layout (weight columns host-permuted): tile (gg, j) has
            # partition 32*s+p <-> head 4*gg+s, dh j*32+p. Scores for head s
            # run as K=32 DoubleRow matmuls on partition quarter s (row
            # placement composes with DR; col placement does not).
            # v8b holds V in block-diagonal head-pair blocks:
            # [:, tok-chunk, pair g, 0, 0:64]=v_{2g}, [:, .., 1, 64:128]=
            # v_{2g+1}, zeros elsewhere -> one M=128 DoubleRow matmul per
            # chunk computes both heads of attn@V at PE column group 0
            # (DR outputs must start at column group 0).
            q8 = ar.tile([P, 4, 2, LT], f8, tag="Q8")
            k8 = ar.tile([P, 4, 2, NT], f8, tag="K16")
            v8b = ar.tile([P, KC, KC, 2, P], f8, tag="V8B")
            nc.gpsimd.memset(v8b[:], 0.0)
            for mt in range(KC):  # q, local tokens
                pq = pp()
                for kp in range(0, KC, 2):
                    nc.tensor.matmul(pq[:], qkA[:, kp:kp + 2, mt * P:(mt + 1) * P],
                                     y8[:, kp:kp + 2, 0:LT],
                                     start=(kp == 0), stop=(kp == KC - 2),
                                     perf_mode=DR)
                nc.scalar.activation(q8[:, mt // 2, mt % 2, :], pq[:],
                                     AF.Identity, bias=qkvbt[:, mt:mt + 1],
                                     scale=IWS)
            for mt in range(KC):  # k, all tokens
                for nh in range(2):
                    pk = pp()
                    for kp in range(0, KC, 2):
                        nc.tensor.matmul(
                            pk[:], qkA[:, kp:kp + 2, D + mt * P:D + (mt + 1) * P],
                            y8[:, kp:kp + 2, nh * 512:(nh + 1) * 512],
                            start=(kp == 0), stop=(kp == KC - 2),
                            perf_mode=DR)
                    nc.scalar.activation(
                        k8[:, mt // 2, mt % 2, nh * 512:(nh + 1) * 512],
                        pk[:], AF.Identity, bias=qkvbt[:, 8 + mt:9 + mt],
                        scale=IWS)
            pb = pbig()  # v bias replicated across partitions
            for nh in range(2):
                nc.tensor.matmul(pb[:, nh * 512:(nh + 1) * 512], ones16[0:1, :],
                                 bvt[:, nh * 512:(nh + 1) * 512],
                                 start=True, stop=True, skip_group_check=True)
            bvrep = rot.tile([P, KC, P], f16, tag="BV", bufs=1, name="bvrep")
            nc.vector.tensor_copy(bvrep[:], pb[:])
            for tt in range(KC):  # v rows = tokens (all)
                pv = psB.tile([P, KC, P], f32, tag="big", name="pv")
                for kp in range(0, KC, 2):
                    for nh in range(2):
                        nc.tensor.matmul(
                            pv[:, nh * 4:(nh + 1) * 4, :],
                            y8[:, kp:kp + 2, tt * P:(tt + 1) * P],
                            vW[:, kp:kp + 2, nh * 512:(nh + 1) * 512],
                            start=(kp == 0), stop=(kp == KC - 2),
                            perf_mode=DR, skip_group_check=True)
                # scatter even/odd head features into the diagonal blocks
                nc.vector.scalar_tensor_tensor(v8b[:, tt, :, 0, 0:DH],
                                               pv[:, :, 0:DH], IWS,
                                               bvrep[:, :, 0:DH],
                                               AluOpType.mult, AluOpType.add)
                nc.vector.scalar_tensor_tensor(v8b[:, tt, :, 1, DH:P],
                                               pv[:, :, DH:P], IWS,
                                               bvrep[:, :, DH:P],
                                               AluOpType.mult, AluOpType.add)

            # ---- attention, head pair (2g, 2g+1) per feature tile g ----
            attn8 = ar.tile([P, KC, LT], f8, tag="AT8")
            pw = ar.tile([P, KC, D], f8, tag="PW")
            nc.sync.dma_start(pw[:], projw8.rearrange("(c p) m -> p c m", p=P))
            for gg in range(4):  # 4 heads 4*gg..4*gg+3 per iteration
                eg = ar.tile([P, KC, 4, 512], f8, tag="EG", bufs=2)
                for c in range(KC):
                    for pr in range(2):  # head pair (4gg+2pr, 4gg+2pr+1)
                        psc = pbig()
                        for par in range(2):
                            s = 2 * pr + par
                            tp = (96, 0) if s == 3 else None
                            nc.tensor.matmul(
                                psc[:, par * 512:(par + 1) * 512],
                                k8[32 * s:32 * (s + 1), gg, :, c * P:(c + 1) * P],
                                q8[32 * s:32 * (s + 1), gg, :, :],
                                start=True, stop=True, perf_mode=DR,
                                skip_group_check=True, tile_position=tp)
                        if c >= 6:
                            # fast-exp on DVE: ACT is the bottleneck here
                            ei = rot.tile([P, NT], dt.int32, tag="EI",
                                          bufs=2, name="ei")
                            nc.vector.tensor_scalar(ei[:], psc[:],
                                                    SCH_A, SCH_B,
                                                    AluOpType.mult,
                                                    AluOpType.add)
                            nc.vector.tensor_copy(
                                eg[:, c, 2 * pr:2 * pr + 2, :],
                                ei[:].bitcast(dt.float32))
                        else:
                            nc.scalar.activation(eg[:, c, 2 * pr:2 * pr + 2, :],
                                                 psc[:], AF.Exp,
                                                 scale=EXP_SCALE, bias=expb[:])
                for pr in range(2):
                    g = 2 * gg + pr  # attn8 feature chunk / head pair index
                    pse = pp()
                    for c in range(KC):
                        nc.tensor.matmul(pse[:], ones8m[:],
                                         eg[:, c, 2 * pr:2 * pr + 2, :],
                                         start=(c == 0), stop=(c == KC - 1),
                                         perf_mode=DR,
                                         skip_group_check=True)
                    recip = rot.tile([P, LT], f16, tag="RC", bufs=2,
                                     name="recip")
                    with nc.allow_low_precision(reason="softmax recip f16"):
                        nc.vector.reciprocal(recip[:], pse[:])
                    pav = pp()
                    for c in range(KC):
                        nc.tensor.matmul(pav[:], v8b[:, c, g],
                                         eg[:, c, 2 * pr:2 * pr + 2, :],
                                         start=(c == 0), stop=(c == KC - 1),
                                         perf_mode=DR)
                    nc.vector.tensor_tensor(attn8[:, g], pav[:],
                                            recip[:], AluOpType.mult)

            # ---- proj + gated residual ----
            f1a = ar.tile([P, KC, 2 * D], f8, tag="W2")  # reuses qkA slot
            nc.sync.dma_start(f1a[:],
                              fc1w8[:, 0:2 * D].rearrange("(c p) m -> p c m", p=P))
            x2 = ar.tile([P, KC, LT], f16, tag="Q16")    # reuses q16 slot
            for mt in range(KC):
                pj = pp()
                for kp in range(0, KC, 2):
                    nc.tensor.matmul(pj[:], pw[:, kp:kp + 2, mt * P:(mt + 1) * P],
                                     attn8[:, kp:kp + 2, :],
                                     start=(kp == 0), stop=False,
                                     perf_mode=DR)
                nc.tensor.matmul(pj[:], pbrow[:, mt * P:(mt + 1) * P],
                                 onesrow[:], start=False, stop=True)
                nc.vector.scalar_tensor_tensor(x2[:, mt], pj[:],
                                               vecs[:, 2, mt:mt + 1],
                                               xf[:, mt, 0:LT],
                                               AluOpType.mult, AluOpType.add)

            # ---- LN2 + modulate (local tokens) -> z8 ----
            z8 = ar.tile([P, KC, LT], f8, tag="AT8")     # reuses attn8 slot
            pss2 = pbig()
            psq2 = pbig()
            layernorm(x2, 0, LT, 3, 4, 7, z8, "STc", pss2, psq2)

            # ---- fc1 + gelu (DoubleRow fp8) ----
            h8 = ar.tile([P, 32, LT], f8, tag="X16")     # reuses xf slot
            f1b = ar.tile([P, KC, 2 * D], f8, tag="K16")  # reuses k16 slot
            nc.sync.dma_start(f1b[:],
                              fc1w8[:, 2 * D:4 * D].rearrange("(c p) m -> p c m", p=P))

            def fc1_block(wt, mg0, nmt):
                for mt in range(nmt):
                    mg = mg0 + mt
                    ph = pp()
                    for kp in range(0, KC, 2):
                        nc.tensor.matmul(ph[:], wt[:, kp:kp + 2, mt * P:(mt + 1) * P],
                                         z8[:, kp:kp + 2, :],
                                         start=(kp == 0), stop=(kp == KC - 2),
                                         perf_mode=DR)
                    nc.scalar.activation(h8[:, mg], ph[:], AF.Gelu,
                                         bias=fc1bt[:, mg:mg + 1], scale=IWS)

            fc1_block(f1a, 0, 16)
            fc1_block(f1b, 16, 16)

            # ---- fc2 + gated residual + store (DoubleRow fp8) ----
            for mt in range(KC):
                f2col = ar.tile([P, 32, P], f8, tag="F2C", bufs=3)
                nc.sync.dma_start(
                    f2col[:],
                    fc2w8[mt * P:(mt + 1) * P, :]
                    .rearrange("p (c m) -> p c m", m=P))
                pz = pp()
                for kp in range(0, 32, 2):
                    nc.tensor.matmul(pz[:], f2col[:, kp:kp + 2, :],
                                     h8[:, kp:kp + 2, :],
                                     start=(kp == 0), stop=False,
                                     perf_mode=DR)
                nc.tensor.matmul(pz[:], f2brow[:, mt * P:(mt + 1) * P],
                                 onesrow[:], start=False, stop=True)
                ot = rot.tile([P, LT], f32, tag="OT", bufs=2)
                nc.vector.scalar_tensor_tensor(ot[:], pz[:],
                                               vecs[:, 5, mt:mt + 1],
                                               x2[:, mt, :],
                                               AluOpType.mult, AluOpType.add)
                nc.sync.dma_start(outt[mt * P:(mt + 1) * P, :], ot[:])

    _legalize_waits(nc)
    return nc


_NC_CACHE = {}


def _get_nc():
    if "nc" not in _NC_CACHE:
        _NC_CACHE["nc"] = _build()
    return _NC_CACHE["nc"]


def _feat(v, cols):
    """[D*]-vector -> feature-major [128, cols] (col j = chunk j)."""
    return np.ascontiguousarray(np.asarray(v, np.float32).reshape(cols, P).T)


def make_in_maps(x, cond, g1_w, g1_b, b1_w, b1_b, a1_w, a1_b,
                 g2_w, g2_b, b2_w, b2_b, a2_w, a2_b,
                 ln1_g, ln1_b, ln2_g, ln2_b,
                 qkv_w, qkv_b, proj_w, proj_b,
                 fc1_w, fc1_b, fc2_w, fc2_b):
    f32 = np.float32
    f16 = np.float16
    f8 = dt.np(dt.float8e4)
    x = np.asarray(x, f32)
    cond = np.asarray(cond, f32)

    def w8(w):
        return (np.asarray(w, f32) * WS).astype(f8)

    # q/k column permutation: tile mt=(gg,j), col p -> head 4*gg+p//32,
    # dh j*32+p%32 (scores run as K=32 DoubleRow on partition quarters)
    permqk = np.empty(D, np.int64)
    pcol = np.arange(P)
    for mt in range(KC):
        gg, j = mt // 2, mt % 2
        permqk[mt * P + pcol] = (4 * gg + pcol // 32) * 64 + j * 32 + (pcol % 32)
    qkv_w = np.asarray(qkv_w, f32)
    qkv_b = np.asarray(qkv_b, f32)
    qkvw_perm = np.concatenate([qkv_w[:, permqk], qkv_w[:, D + permqk],
                                qkv_w[:, 2 * D:3 * D]], axis=1)

    shared = {
        "qkvw8": w8(qkvw_perm),
        "qkvbf": np.hstack([_feat(qkv_b[permqk], KC),
                            _feat(qkv_b[D + permqk], KC)]),
        "bvrow": np.asarray(qkv_b, f16)[None, 2 * D:3 * D],
        "projw8": w8(proj_w),
        "projbrow": (np.asarray(proj_b, f32) * WS).astype(f16)[None, :],
        "fc1w8": w8(fc1_w),
        "fc1bf": _feat(np.asarray(fc1_b, f32), 32),
        # [mt*128+p, kc*128+m] = fc2_w[kc*128+p, mt*128+m]: contiguous
        # per-mt loads of the feature-major lhsT tiles
        "fc2w8": np.ascontiguousarray(
            w8(fc2_w).reshape(32, P, KC, P)
            .transpose(2, 1, 0, 3).reshape(D, DFF)),
        "fc2brow": (np.asarray(fc2_b, f32) * WS).astype(f16)[None, :],
    }

    # host-folded modulation vectors (25M MACs, 0.02% of model FLOPs)
    g1 = cond @ np.asarray(g1_w, f32) + np.asarray(g1_b, f32)
    b1 = cond @ np.asarray(b1_w, f32) + np.asarray(b1_b, f32)
    a1 = np.tanh(cond @ np.asarray(a1_w, f32) + np.asarray(a1_b, f32)) * GATE
    g2 = cond @ np.asarray(g2_w, f32) + np.asarray(g2_b, f32)
    b2 = cond @ np.asarray(b2_w, f32) + np.asarray(b2_b, f32)
    a2 = np.tanh(cond @ np.asarray(a2_w, f32) + np.asarray(a2_b, f32)) * GATE
    s1 = (1.0 + g1) * np.asarray(ln1_g, f32)[None, :]
    t1 = (1.0 + g1) * np.asarray(ln1_b, f32)[None, :] + b1
    s2 = (1.0 + g2) * np.asarray(ln2_g, f32)[None, :]
    t2 = (1.0 + g2) * np.asarray(ln2_b, f32)[None, :] + b2

    in_maps = []
    for c in range(8):
        b, h = c // 2, c % 2
        xb = x[b].T  # [D, NT]
        perm = np.concatenate([np.arange(h * LT, (h + 1) * LT),
                               np.arange((1 - h) * LT, (2 - h) * LT)])
        m = dict(shared)
        m["xt16"] = np.ascontiguousarray(xb[:, perm]).astype(f16)
        m["vecsf"] = np.hstack([
            _feat(s1[b], KC), _feat(t1[b], KC), _feat(a1[b] * IWS, KC),
            _feat(s2[b], KC), _feat(t2[b], KC), _feat(a2[b] * IWS, KC),
            _feat(-s1[b], KC), _feat(-s2[b], KC)])
        in_maps.append(m)
    return in_maps


def kernel(**inputs):
    nc = _get_nc()
    in_maps = make_in_maps(**inputs)
    res = run_bass_kernel_spmd(nc, in_maps, list(range(8)))
    out = np.empty((B, NT, D), np.float32)
    for c in range(8):
        b, h = c // 2, c % 2
        out[b, h * LT:(h + 1) * LT, :] = res.results[c]["outt"].T
    return out


# revision 44
# speedup vs baseline: 2.8234x; 2.0635x over previous
"""DiT block kernel for 8 trn2 NeuronCores.

Sharding: core c -> (batch b=c//2, query-token half h=c%2). Each core
computes the full block for its 512 query tokens (K/V for all 1024
tokens of its batch replicated within the pair) -> zero collectives.

v2: fp8e4 + DoubleRow matmuls for qkv/proj/fc1/fc2/softmax-sums/attnV
(weights host-scaled by WS=32, descaled at PSUM evacuation); scores stay
fp16; LN stats in fp16; modulation vectors folded on the host into the
input prep (tiny matvecs, 0.02% of model FLOPs). Activations are
feature-major ([D on partitions, tokens on free]).
"""
import numpy as np

import concourse.bass as bass
import concourse.tile as tile
import concourse.mybir as mybir
from concourse.bass_utils import run_bass_kernel_spmd
from concourse.vector_clock import ScopedClock
from concourse.alu_op_type import AluOpType

dt = mybir.dt
AF = mybir.ActivationFunctionType
DR = mybir.MatmulPerfMode.DoubleRow

P = 128
B, NT, D, H = 4, 1024, 1024, 16
DH = D // H            # 64
DFF = 4 * D            # 4096
KC = D // P            # 8
LT = NT // 2           # 512 local query tokens
GATE = 0.1
EPS = 1e-5
EXP_SCALE = DH ** -0.5
EXP_BIAS = -1.5        # keeps exp outputs inside fp8e4m3 range; cancels
WS = 32.0              # fp8 weight pre-scale
IWS = 1.0 / WS
# Schraudolph fast-exp (DVE bit-trick): i32 = A*scores + B stays positive
# over the whole score range (B ~ 1.05e9), bitcast to f32 approximates
# exp(EXP_SCALE*scores + EXP_BIAS) to ~2-4% (~ fp8 quantization anyway).
SCH_A = 8388608.0 * 1.4426950408889634 * EXP_SCALE
SCH_B = 8388608.0 * (127.0 + 1.4426950408889634 * EXP_BIAS) - 366393.0


class SplitDrainTileContext(tile.TileContext):
    """Tail drain in this walrus build holds few sync waits; spill the
    rest onto chained SP nops (runs before the sem-clear barrier, so
    semantics are preserved)."""

    MAX_TAIL_WAITS = 1

    def _drain_and_barrier(self, tick_clock, wait_clock):
        drain_inst = self.nc.sync.drain()
        wait_clock.add_sem_waits(
            drain_inst.ins, ScopedClock({None: tick_clock.global_clock})
        )
        si = drain_inst.ins.sync_info
        waits = list(si.on_wait) if si else []
        if len(waits) > self.MAX_TAIL_WAITS:
            drain_inst.ins.sync_info = mybir.SyncInfo(
                on_wait=waits[: self.MAX_TAIL_WAITS],
                on_update=list(si.on_update) if si else [],
            )
            rest = waits[self.MAX_TAIL_WAITS:]
            for i in range(0, len(rest), self.MAX_TAIL_WAITS):
                nop = self.nc.sync.nop()
                nop.ins.sync_info = mybir.SyncInfo(
                    on_wait=rest[i : i + self.MAX_TAIL_WAITS], on_update=[]
                )
        self.nc.all_engine_barrier()
        assert self.sems is not None
        popped = self.nc._tile_sem_poison_stack.pop()
        assert popped is self._sem_poison
        self.nc.clear_and_free_semaphores(list(self.sems.allocated().values()))
        self.nc.all_engine_barrier()


def _legalize_waits(nc, max_waits=1):
    """This walrus build accepts at most one sync wait per instruction.
    Move surplus waits onto same-engine NoOps inserted just before the
    offending instruction (engine FIFO order preserves semantics)."""
    fix = 0
    for bb in nc.main_func.blocks:
        insts = list(bb.instructions)
        out = []
        for inst in insts:
            si = inst.sync_info
            waits = list(si.on_wait) if si else []
            if len(waits) > max_waits:
                keep = waits[-max_waits:]
                for w in waits[:-max_waits]:
                    nop = mybir.InstNoOp(name=f"I-wfix{fix}")
                    fix += 1
                    nop.engine = inst.engine
                    nop.sync_info = mybir.SyncInfo(on_wait=[w], on_update=[])
                    out.append(nop)
                inst.sync_info = mybir.SyncInfo(
                    on_wait=keep, on_update=list(si.on_update) if si else [])
            out.append(inst)
        if len(out) != len(insts):
            bb.instructions = out
    return fix


def _build(exp_dve_c=7, ln1h1_pool=True, ln2_pool=False, nchain=1):
    nc = bass.Bass(target_bir_lowering=False, debug=False,
                   dynamic_dma_scratch_size=2048)
    f32, f16, f8 = dt.float32, dt.float16, dt.float8e4

    xt16 = nc.dram_tensor("xt16", [D, NT], f16, kind="ExternalInput")
    vecsf = nc.dram_tensor("vecsf", [P, 8 * KC], f32, kind="ExternalInput")
    qkvw8 = nc.dram_tensor("qkvw8", [D, 3 * D], f8, kind="ExternalInput")
    qkvbf = nc.dram_tensor("qkvbf", [P, 16], f32, kind="ExternalInput")
    bvrow = nc.dram_tensor("bvrow", [1, D], f16, kind="ExternalInput")
    projw8 = nc.dram_tensor("projw8", [D, D], f8, kind="ExternalInput")
    projbrow = nc.dram_tensor("projbrow", [1, D], f16, kind="ExternalInput")
    fc1w8 = nc.dram_tensor("fc1w8", [D, DFF], f8, kind="ExternalInput")
    fc1bf = nc.dram_tensor("fc1bf", [P, 32], f32, kind="ExternalInput")
    fc2w8 = nc.dram_tensor("fc2w8", [D, DFF], f8, kind="ExternalInput")  # host-rearranged
    fc2brow = nc.dram_tensor("fc2brow", [1, D], f16, kind="ExternalInput")
    outt = nc.dram_tensor("outt", [D, LT], f32, kind="ExternalOutput")

    with SplitDrainTileContext(nc) as tc:
        with tc.tile_pool(name="cp", bufs=1) as cp, \
             tc.tile_pool(name="ar", bufs=1) as ar, \
             tc.tile_pool(name="rot", bufs=4) as rot, \
             tc.tile_pool(name="psA", bufs=2, space="PSUM") as psA, \
             tc.tile_pool(name="psB", bufs=3, space="PSUM") as psB:

            def pp():    # [P, 512] f32 psum, 2 rotating banks
                return psA.tile([P, 512], f32, tag="pp", name="pp")

            def pbig():  # [P, 1024] f32 psum, 3 rotating 2-bank tiles
                return psB.tile([P, 1024], f32, tag="big", name="big")

            ones16 = cp.tile([P, P], f16, tag="ones16")
            nc.vector.memset(ones16[:], 1.0)
            ones8 = cp.tile([P, 2, P], f8, tag="ones8")
            nc.vector.memset(ones8[:], 1.0)
            # masked ones: slot0 hits out cols 0:64, slot1 cols 64:128 ->
            # softmax DR row-sums land per-head in one [128, 512] tile
            # (rows 0:64 = even head, 64:128 = odd head)
            ones8m = cp.tile([P, 2, P], f8, tag="ones8m")
            nc.vector.memset(ones8m[:], 1.0)
            nc.vector.memset(ones8m[:, 0, DH:P], 0.0)
            nc.vector.memset(ones8m[:, 1, 0:DH], 0.0)
            onesrow = cp.tile([1, LT], f16, tag="onesrow")
            nc.vector.memset(onesrow[:], 1.0)
            expb = cp.tile([P, 1], f32, tag="expb")
            nc.vector.memset(expb[:], EXP_BIAS)

            # ---- resident small inputs ----
            vecs = cp.tile([P, 8, KC], f32, tag="vecs")
            nc.sync.dma_start(vecs[:], vecsf.rearrange("p (w c) -> p w c", c=KC))
            qkvbt = cp.tile([P, 16], f32, tag="qkvbt")
            nc.sync.dma_start(qkvbt[:], qkvbf[:])
            fc1bt = cp.tile([P, 32], f32, tag="fc1bt")
            nc.sync.dma_start(fc1bt[:], fc1bf[:])
            bvt = cp.tile([1, D], f16, tag="bvt")
            nc.sync.dma_start(bvt[:], bvrow[:])
            pbrow = cp.tile([1, D], f16, tag="pbrow")
            nc.sync.dma_start(pbrow[:], projbrow[:])
            f2brow = cp.tile([1, D], f16, tag="f2brow")
            nc.sync.dma_start(f2brow[:], fc2brow[:])

            xf = ar.tile([P, KC, NT], f16, tag="X16")
            xsrc = xt16.rearrange("(c p) t -> p c t", p=P)
            nc.sync.dma_start(xf[:, 0:4, 0:512], xsrc[:, 0:4, 0:512])
            nc.sync.dma_start(xf[:, 4:8, 0:512], xsrc[:, 4:8, 0:512])
            nc.sync.dma_start(xf[:, :, 512:1024], xsrc[:, :, 512:1024])
            qkA = ar.tile([P, KC, 2 * D], f8, tag="W2")
            nc.sync.dma_start(qkA[:],
                              qkvw8[:, 0:2 * D].rearrange("(c p) m -> p c m", p=P))
            vW = ar.tile([P, KC, D], f8, tag="VW")
            nc.sync.dma_start(vW[:],
                              qkvw8[:, 2 * D:3 * D].rearrange("(c p) m -> p c m", p=P))

            def layernorm(src16, t0, t1, scol, bcol, ncol, out8, stag,
                          pss, psq, u_engine=None):
                """out8[:, j, t0:t1] = (LN(src)*scale+shift) in fp8e4 for
                token range [t0, t1); f16 stats accumulated in the column
                range [t0, t1) of the shared pss/psq psum tiles."""
                ntok = t1 - t0
                sl = slice(t0, t1)
                for j in range(KC):
                    s16 = rot.tile([P, 512], f16, tag="S16", bufs=3, name="s16")
                    nc.vector.tensor_tensor(s16[:, 0:ntok],
                                            src16[:, j, sl],
                                            src16[:, j, sl],
                                            AluOpType.mult)
                    nc.tensor.matmul(pss[:, sl], ones16[:],
                                     src16[:, j, sl],
                                     start=(j == 0), stop=(j == KC - 1),
                                     skip_group_check=True)
                    nc.tensor.matmul(psq[:, sl], ones16[:],
                                     s16[:, 0:ntok],
                                     start=(j == 0), stop=(j == KC - 1),
                                     skip_group_check=True)

                def st(name):
                    return rot.tile([P, ntok], f16, tag=stag, bufs=7,
                                    name=name)

                mu = st("mu")
                nc.vector.tensor_scalar_mul(mu[:], pss[:, sl], 1.0 / D)
                msq = st("msq")
                nc.vector.tensor_scalar(msq[:], psq[:, sl], 1.0 / D, EPS,
                                        AluOpType.mult, AluOpType.add)
                mu2 = st("mu2")
                nc.vector.tensor_tensor(mu2[:], mu[:], mu[:], AluOpType.mult)
                var = st("var")
                nc.vector.tensor_tensor(var[:], msq[:], mu2[:],
                                        AluOpType.subtract)
                rvar = st("rvar")
                with nc.allow_low_precision(reason="LN rstd fine in f16"):
                    nc.vector.reciprocal(rvar[:], var[:])
                A = st("A")
                nc.scalar.activation(A[:], rvar[:], AF.Sqrt)
                M = st("M")
                nc.vector.tensor_tensor(M[:], mu[:], A[:], AluOpType.mult)
                ueng = u_engine or nc.vector
                for j in range(KC):
                    u = rot.tile([P, 512], f16, tag="U16", bufs=3, name="u")
                    ueng.tensor_tensor(u[:, 0:ntok], src16[:, j, sl],
                                       A[:], AluOpType.mult)
                    w = rot.tile([P, 512], f16, tag="U16", bufs=3, name="w")
                    nc.scalar.activation(w[:, 0:ntok], u[:, 0:ntok],
                                         AF.Identity,
                                         bias=vecs[:, bcol, j:j + 1],
                                         scale=vecs[:, scol, j:j + 1])
                    nc.vector.scalar_tensor_tensor(out8[:, j, sl], M[:],
                                                   vecs[:, ncol, j:j + 1],
                                                   w[:, 0:ntok],
                                                   AluOpType.mult,
                                                   AluOpType.add)

            # ---- LN1 + modulate (all 1024 tokens) -> y8, half-pipelined ----
            y8 = ar.tile([P, KC, NT], f8, tag="Y8")
            pss1 = pbig()
            psq1 = pbig()
            layernorm(xf, 0, 512, 0, 1, 6, y8, "STa", pss1, psq1)
            layernorm(xf, 512, 1024, 0, 1, 6, y8, "STb", pss1, psq1,
                      u_engine=nc.gpsimd if ln1h1_pool else None)

            # ---- qkv (DoubleRow fp8) ----
            # q8/k8 layout (weight columns host-permuted): tile (gg, j) has
            # partition 32*s+p <-> head 4*gg+s, dh j*32+p. Scores for head s
            # run as K=32 DoubleRow matmuls on partition quarter s (row
            # placement composes with DR; col placement does not).
            # v8b holds V in block-diagonal head-pair blocks:
            # [:, tok-chunk, pair g, 0, 0:64]=v_{2g}, [:, .., 1, 64:128]=
            # v_{2g+1}, zeros elsewhere -> one M=128 DoubleRow matmul per
            # chunk computes both heads of attn@V at PE column group 0
            # (DR outputs must start at column group 0).
            q8 = ar.tile([P, 4, 2, LT], f8, tag="Q8")
            k8 = ar.tile([P, 4, 2, NT], f8, tag="K16")
            v8b = ar.tile([P, KC, KC, 2, P], f8, tag="V8B")
            nc.gpsimd.memset(v8b[:], 0.0)
            for mt in range(KC):  # q, local tokens
                pq = pp()
                for kp in range(0, KC, 2):
                    nc.tensor.matmul(pq[:], qkA[:, kp:kp + 2, mt * P:(mt + 1) * P],
                                     y8[:, kp:kp + 2, 0:LT],
                                     start=(kp == 0), stop=(kp == KC - 2),
                                     perf_mode=DR)
                nc.scalar.activation(q8[:, mt // 2, mt % 2, :], pq[:],
                                     AF.Identity, bias=qkvbt[:, mt:mt + 1],
                                     scale=IWS)
            for mt in range(KC):  # k, all tokens
                for nh in range(2):
                    pk = pp()
                    for kp in range(0, KC, 2):
                        nc.tensor.matmul(
                            pk[:], qkA[:, kp:kp + 2, D + mt * P:D + (mt + 1) * P],
                            y8[:, kp:kp + 2, nh * 512:(nh + 1) * 512],
                            start=(kp == 0), stop=(kp == KC - 2),
                            perf_mode=DR)
                    nc.scalar.activation(
                        k8[:, mt // 2, mt % 2, nh * 512:(nh + 1) * 512],
                        pk[:], AF.Identity, bias=qkvbt[:, 8 + mt:9 + mt],
                        scale=IWS)
            pb = pbig()  # v bias replicated across partitions
            for nh in range(2):
                nc.tensor.matmul(pb[:, nh * 512:(nh + 1) * 512], ones16[0:1, :],
                                 bvt[:, nh * 512:(nh + 1) * 512],
                                 start=True, stop=True, skip_group_check=True)
            bvrep = rot.tile([P, KC, P], f16, tag="BV", bufs=1, name="bvrep")
            nc.vector.tensor_copy(bvrep[:], pb[:])
            for tt in range(KC):  # v rows = tokens (all)
                pv = psB.tile([P, KC, P], f32, tag="big", name="pv")
                for kp in range(0, KC, 2):
                    for nh in range(2):
                        nc.tensor.matmul(
                            pv[:, nh * 4:(nh + 1) * 4, :],
                            y8[:, kp:kp + 2, tt * P:(tt + 1) * P],
                            vW[:, kp:kp + 2, nh * 512:(nh + 1) * 512],
                            start=(kp == 0), stop=(kp == KC - 2),
                            perf_mode=DR, skip_group_check=True)
                # scatter even/odd head features into the diagonal blocks
                nc.vector.scalar_tensor_tensor(v8b[:, tt, :, 0, 0:DH],
                                               pv[:, :, 0:DH], IWS,
                                               bvrep[:, :, 0:DH],
                                               AluOpType.mult, AluOpType.add)
                nc.vector.scalar_tensor_tensor(v8b[:, tt, :, 1, DH:P],
                                               pv[:, :, DH:P], IWS,
                                               bvrep[:, :, DH:P],
                                               AluOpType.mult, AluOpType.add)

            # ---- attention, head pair (2g, 2g+1) per feature tile g ----
            attn8 = ar.tile([P, KC, LT], f8, tag="AT8")
            pw = ar.tile([P, KC, D], f8, tag="PW")
            nc.sync.dma_start(pw[:], projw8.rearrange("(c p) m -> p c m", p=P))
            for gg in range(4):  # 4 heads 4*gg..4*gg+3 per iteration
                eg = ar.tile([P, KC, 4, 512], f8, tag="EG", bufs=2)
                for c in range(KC):
                    for pr in range(2):  # head pair (4gg+2pr, 4gg+2pr+1)
                        psc = pbig()
                        for par in range(2):
                            s = 2 * pr + par
                            tp = (96, 0) if s == 3 else None
                            nc.tensor.matmul(
                                psc[:, par * 512:(par + 1) * 512],
                                k8[32 * s:32 * (s + 1), gg, :, c * P:(c + 1) * P],
                                q8[32 * s:32 * (s + 1), gg, :, :],
                                start=True, stop=True, perf_mode=DR,
                                skip_group_check=True, tile_position=tp)
                        if c >= exp_dve_c:
                            # fast-exp on DVE: ACT is the bottleneck here
                            ei = rot.tile([P, NT], dt.int32, tag="EI",
                                          bufs=2, name="ei")
                            nc.vector.tensor_scalar(ei[:], psc[:],
                                                    SCH_A, SCH_B,
                                                    AluOpType.mult,
                                                    AluOpType.add)
                            nc.vector.tensor_copy(
                                eg[:, c, 2 * pr:2 * pr + 2, :],
                                ei[:].bitcast(dt.float32))
                        else:
                            nc.scalar.activation(eg[:, c, 2 * pr:2 * pr + 2, :],
                                                 psc[:], AF.Exp,
                                                 scale=EXP_SCALE, bias=expb[:])
                for pr in range(2):
                    g = 2 * gg + pr  # attn8 feature chunk / head pair index
                    pse = pp()
                    for c in range(KC):
                        nc.tensor.matmul(pse[:], ones8m[:],
                                         eg[:, c, 2 * pr:2 * pr + 2, :],
                                         start=(c == 0), stop=(c == KC - 1),
                                         perf_mode=DR,
                                         skip_group_check=True)
                    recip = rot.tile([P, LT], f16, tag="RC", bufs=2,
                                     name="recip")
                    with nc.allow_low_precision(reason="softmax recip f16"):
                        nc.vector.reciprocal(recip[:], pse[:])
                    pav = pp()
                    for c in range(KC):
                        nc.tensor.matmul(pav[:], v8b[:, c, g],
                                         eg[:, c, 2 * pr:2 * pr + 2, :],
                                         start=(c == 0), stop=(c == KC - 1),
                                         perf_mode=DR)
                    nc.vector.tensor_tensor(attn8[:, g], pav[:],
                                            recip[:], AluOpType.mult)

            # ---- proj + gated residual ----
            f1a = ar.tile([P, KC, 2 * D], f8, tag="W2")  # reuses qkA slot
            nc.sync.dma_start(f1a[:],
                              fc1w8[:, 0:2 * D].rearrange("(c p) m -> p c m", p=P))
            x2 = ar.tile([P, KC, LT], f16, tag="Q16")    # reuses q16 slot

            def chained_mm(n_out, n_kp, lhsT_of, rhs_of, brow, evac):
                """n_out accumulation chains interleaved kp-major so the PE
                stream follows the producer of rhs chunks instead of
                serializing whole chains. Chains borrow psum: 2 pp tiles +
                halves of 2 pbig tiles = 6 concurrent chains per round."""
                for r0 in range(0, n_out, nchain):
                    mts = range(r0, min(r0 + nchain, n_out))
                    pzs = {}
                    for i, mt in enumerate(mts):
                        if i < 2:
                            pzs[mt] = pp()
                        else:
                            j = i - 2
                            if j % 2 == 0:
                                big = pbig()
                            pzs[mt] = big[:, (j % 2) * 512:(j % 2 + 1) * 512]
                    for kp in range(0, n_kp, 2):
                        for mt in mts:
                            nc.tensor.matmul(pzs[mt], lhsT_of(mt, kp),
                                             rhs_of(mt, kp),
                                             start=(kp == 0), stop=False,
                                             perf_mode=DR,
                                             skip_group_check=True)
                    for mt in mts:
                        nc.tensor.matmul(pzs[mt],
                                         brow[:, mt * P:(mt + 1) * P],
                                         onesrow[:], start=False, stop=True,
                                         skip_group_check=True)
                        evac(mt, pzs[mt])

            chained_mm(
                KC, KC,
                lambda mt, kp: pw[:, kp:kp + 2, mt * P:(mt + 1) * P],
                lambda mt, kp: attn8[:, kp:kp + 2, :],
                pbrow,
                lambda mt, pj: nc.vector.scalar_tensor_tensor(
                    x2[:, mt], pj[:], vecs[:, 2, mt:mt + 1],
                    xf[:, mt, 0:LT], AluOpType.mult, AluOpType.add))

            # ---- LN2 + modulate (local tokens) -> z8 ----
            z8 = ar.tile([P, KC, LT], f8, tag="AT8")     # reuses attn8 slot
            pss2 = pbig()
            psq2 = pbig()
            layernorm(x2, 0, LT, 3, 4, 7, z8, "STc", pss2, psq2,
                      u_engine=nc.gpsimd if ln2_pool else None)

            # ---- fc1 + gelu (DoubleRow fp8) ----
            h8 = ar.tile([P, 32, LT], f8, tag="X16")     # reuses xf slot
            f1b = ar.tile([P, KC, 2 * D], f8, tag="K16")  # reuses k16 slot
            nc.sync.dma_start(f1b[:],
                              fc1w8[:, 2 * D:4 * D].rearrange("(c p) m -> p c m", p=P))

            def fc1_block(wt, mg0, nmt):
                for mt in range(nmt):
                    mg = mg0 + mt
                    ph = pp()
                    for kp in range(0, KC, 2):
                        nc.tensor.matmul(ph[:], wt[:, kp:kp + 2, mt * P:(mt + 1) * P],
                                         z8[:, kp:kp + 2, :],
                                         start=(kp == 0), stop=(kp == KC - 2),
                                         perf_mode=DR)
                    nc.scalar.activation(h8[:, mg], ph[:], AF.Gelu,
                                         bias=fc1bt[:, mg:mg + 1], scale=IWS)

            fc1_block(f1a, 0, 16)
            fc1_block(f1b, 16, 16)

            # ---- fc2 + gated residual + store (DoubleRow fp8) ----
            f2cols = []
            for mt in range(KC):
                f2col = ar.tile([P, 32, P], f8, tag="F2C", bufs=8)
                nc.sync.dma_start(
                    f2col[:],
                    fc2w8[mt * P:(mt + 1) * P, :]
                    .rearrange("p (c m) -> p c m", m=P))
                f2cols.append(f2col)

            def fc2_evac(mt, pz):
                ot = rot.tile([P, LT], f32, tag="OT", bufs=2)
                nc.vector.scalar_tensor_tensor(ot[:], pz[:],
                                               vecs[:, 5, mt:mt + 1],
                                               x2[:, mt, :],
                                               AluOpType.mult, AluOpType.add)
                # out stores ride the ACT HWDGE queue so the f2col weight
                # prefetches on the SP queue aren't head-of-line blocked
                nc.scalar.dma_start(outt[mt * P:(mt + 1) * P, :], ot[:])

            chained_mm(
                KC, 32,
                lambda mt, kp: f2cols[mt][:, kp:kp + 2, :],
                lambda mt, kp: h8[:, kp:kp + 2, :],
                f2brow, fc2_evac)

    _legalize_waits(nc)
    return nc


_NC_CACHE = {}


def _get_nc():
    if "nc" not in _NC_CACHE:
        _NC_CACHE["nc"] = _build()
    return _NC_CACHE["nc"]


def _feat(v, cols):
    """[D*]-vector -> feature-major [128, cols] (col j = chunk j)."""
    return np.ascontiguousarray(np.asarray(v, np.float32).reshape(cols, P).T)


def make_in_maps(x, cond, g1_w, g1_b, b1_w, b1_b, a1_w, a1_b,
                 g2_w, g2_b, b2_w, b2_b, a2_w, a2_b,
                 ln1_g, ln1_b, ln2_g, ln2_b,
                 qkv_w, qkv_b, proj_w, proj_b,
                 fc1_w, fc1_b, fc2_w, fc2_b):
    f32 = np.float32
    f16 = np.float16
    f8 = dt.np(dt.float8e4)
    x = np.asarray(x, f32)
    cond = np.asarray(cond, f32)

    def w8(w):
        return (np.asarray(w, f32) * WS).astype(f8)

    # q/k column permutation: tile mt=(gg,j), col p -> head 4*gg+p//32,
    # dh j*32+p%32 (scores run as K=32 DoubleRow on partition quarters)
    permqk = np.empty(D, np.int64)
    pcol = np.arange(P)
    for mt in range(KC):
        gg, j = mt // 2, mt % 2
        permqk[mt * P + pcol] = (4 * gg + pcol // 32) * 64 + j * 32 + (pcol % 32)
    qkv_w = np.asarray(qkv_w, f32)
    qkv_b = np.asarray(qkv_b, f32)
    qkvw_perm = np.concatenate([qkv_w[:, permqk], qkv_w[:, D + permqk],
                                qkv_w[:, 2 * D:3 * D]], axis=1)

    shared = {
        "qkvw8": w8(qkvw_perm),
        "qkvbf": np.hstack([_feat(qkv_b[permqk], KC),
                            _feat(qkv_b[D + permqk], KC)]),
        "bvrow": np.asarray(qkv_b, f16)[None, 2 * D:3 * D],
        "projw8": w8(proj_w),
        "projbrow": (np.asarray(proj_b, f32) * WS).astype(f16)[None, :],
        "fc1w8": w8(fc1_w),
        "fc1bf": _feat(np.asarray(fc1_b, f32), 32),
        # [mt*128+p, kc*128+m] = fc2_w[kc*128+p, mt*128+m]: contiguous
        # per-mt loads of the feature-major lhsT tiles
        "fc2w8": np.ascontiguousarray(
            w8(fc2_w).reshape(32, P, KC, P)
            .transpose(2, 1, 0, 3).reshape(D, DFF)),
        "fc2brow": (np.asarray(fc2_b, f32) * WS).astype(f16)[None, :],
    }

    # host-folded modulation vectors (25M MACs, 0.02% of model FLOPs)
    g1 = cond @ np.asarray(g1_w, f32) + np.asarray(g1_b, f32)
    b1 = cond @ np.asarray(b1_w, f32) + np.asarray(b1_b, f32)
    a1 = np.tanh(cond @ np.asarray(a1_w, f32) + np.asarray(a1_b, f32)) * GATE
    g2 = cond @ np.asarray(g2_w, f32) + np.asarray(g2_b, f32)
    b2 = cond @ np.asarray(b2_w, f32) + np.asarray(b2_b, f32)
    a2 = np.tanh(cond @ np.asarray(a2_w, f32) + np.asarray(a2_b, f32)) * GATE
    s1 = (1.0 + g1) * np.asarray(ln1_g, f32)[None, :]
    t1 = (1.0 + g1) * np.asarray(ln1_b, f32)[None, :] + b1
    s2 = (1.0 + g2) * np.asarray(ln2_g, f32)[None, :]
    t2 = (1.0 + g2) * np.asarray(ln2_b, f32)[None, :] + b2

    in_maps = []
    for c in range(8):
        b, h = c // 2, c % 2
        xb = x[b].T  # [D, NT]
        perm = np.concatenate([np.arange(h * LT, (h + 1) * LT),
                               np.arange((1 - h) * LT, (2 - h) * LT)])
        m = dict(shared)
        m["xt16"] = np.ascontiguousarray(xb[:, perm]).astype(f16)
        m["vecsf"] = np.hstack([
            _feat(s1[b], KC), _feat(t1[b], KC), _feat(a1[b] * IWS, KC),
            _feat(s2[b], KC), _feat(t2[b], KC), _feat(a2[b] * IWS, KC),
            _feat(-s1[b], KC), _feat(-s2[b], KC)])
        in_maps.append(m)
    return in_maps


def kernel(**inputs):
    nc = _get_nc()
    in_maps = make_in_maps(**inputs)
    res = run_bass_kernel_spmd(nc, in_maps, list(range(8)))
    out = np.empty((B, NT, D), np.float32)
    for c in range(8):
        b, h = c // 2, c % 2
        out[b, h * LT:(h + 1) * LT, :] = res.results[c]["outt"].T
    return out
